# revision 14
# baseline (speedup 1.0000x reference)
"""ColBERT MaxSim kernel for 8 Trainium2 NeuronCores (Bass/Tile).

Math (matches the reference):
  Q  = l2norm(q_hidden @ W^T)                       (64, 32, 128)
  D  = l2norm(d_hidden @ W^T), masked tokens zeroed (512, 256, 128)
  sim[b,n,q,d] = Q[b] @ D[b*8+n]^T ; masked -> -inf
  out[b,n] = mean_q max_d sim                       (64, 8)

Sharding: data-parallel over the query-group dim B=64 -> 8 groups per
core; each core also owns the matching 64 docs (doc g belongs to group
g//8). W is replicated. No cross-core communication.

Device layout: "features/hidden on partitions". Each core receives its
d/q shards pre-transposed to [768, tokens] (host-side relayout during
sharding) so every DMA is contiguous-per-partition and every matmul has
the contraction dim on partitions. The pad/skiplist mask is folded in by
accumulating +1e30 * antimask into the squared-norm sums (masked tokens
then get inv_norm ~ 1e-15, i.e. D columns ~ 0, which never win the max:
true maxima of these cosine sims are > 0; checked in test.py).
"""

import sys

sys.path.insert(0, "/opt/trn_rl_repo")

from contextlib import ExitStack

import ml_dtypes
import numpy as np

import concourse.bass as bass
import concourse.tile as tile
from concourse import bacc, mybir
from concourse.bass import ts, ds
from concourse.bass_utils import run_bass_kernel_spmd

B_Q, L_Q = 64, 32
B_D, L_D = 512, 256
HID, OUT = 768, 128
N_CORES = 8

GROUPS = B_Q // N_CORES            # 8 query groups per core
N_P = B_D // B_Q                   # 8 docs per group
DTOK = GROUPS * N_P * L_D          # 16384 doc tokens per core
QTOK = GROUPS * L_Q                # 256 query tokens per core
K_CH = HID // 128                  # 6 contraction chunks
TN = 512                           # doc tokens per tile
D_TILES = DTOK // TN               # 32
TILES_PER_G = (N_P * L_D) // TN    # 4 tiles per query group
BIG = 1.0e30
F32 = mybir.dt.float32
BF16 = mybir.dt.bfloat16


def _build_program(f32r_proj=False, f32r_s2=False, f32r_sim=False, reps=1,
                   loop_reps=None, trace_sim=False, dma_only=False,
                   dma_tile=1):
    """Build + compile the per-core Bass program. Returns the Bacc instance.

    reps: python-unrolled repetitions of the whole pipeline (timing only).
    loop_reps: if set, wrap the pipeline in a hardware For_i loop with this
      trip count instead (timing only; smaller program).
    """
    nc = bacc.Bacc("TRN2", target_bir_lowering=False, debug=False,
                   num_devices=N_CORES)

    # tiled host layouts: one doc tile = [128 part, 6 kchunk, 512 tok]
    # contiguous in DRAM (12KB per partition per tile), for line-rate DMA
    dT = nc.dram_tensor("dT", [D_TILES, 128, K_CH, TN], F32,
                        kind="ExternalInput").ap()
    qT = nc.dram_tensor("qT", [128, K_CH, QTOK], F32,
                        kind="ExternalInput").ap()
    wT = nc.dram_tensor("wT", [128, K_CH, OUT], F32,
                        kind="ExternalInput").ap()
    am = nc.dram_tensor("am", [1, DTOK], BF16, kind="ExternalInput").ap()
    out = nc.dram_tensor("out", [1, GROUPS * N_P], F32,
                         kind="ExternalOutput").ap()

    # float32r tiles run matmuls at 4x the fp32 rate; the walrus verifier
    # requires every producer of an fp32r-matmul operand to emit fp32r, so
    # the dtype is set on the tiles (and DRAM-side APs are bitcast).
    F32R = mybir.dt.float32r
    PDT = F32R if f32r_proj else F32   # projection operands (wt, dx, qx)
    SDT = F32R if f32r_s2 else F32     # squared-sum operands (ones128, dsq)
    MDT = F32R if f32r_sim else F32    # maxsim operands (Qn, Dn)

    def rp(ap):
        return ap.bitcast(F32R) if f32r_proj else ap

    dT4 = rp(dT)                                        # [32, 128, 6, 512]
    qT3 = rp(qT)                                        # [128, 6, 256]
    wT3 = rp(wT)                                        # [128, 6, 128]

    with tile.TileContext(nc, trace_sim=trace_sim) as tc, ExitStack() as ctx:
        const = ctx.enter_context(tc.tile_pool(name="const", bufs=1))
        persist = ctx.enter_context(tc.tile_pool(name="persist", bufs=1))
        sb = ctx.enter_context(tc.tile_pool(name="sb", bufs=2))
        qsb = ctx.enter_context(tc.tile_pool(name="qsb", bufs=1))

        wt = const.tile([128, K_CH, OUT], PDT)
        nc.sync.dma_start(out=wt[:], in_=wT3[:, :, :])
        amrow = const.tile([1, DTOK], BF16)
        nc.sync.dma_start(out=amrow[:], in_=am[:, :])
        ones128 = const.tile([128, 128], SDT)
        nc.vector.memset(ones128[:], 1.0)
        onesbig = const.tile([1, 128], BF16)
        nc.vector.memset(onesbig[:], BIG)
        ones32 = const.tile([32, 1], F32)
        nc.vector.memset(ones32[:], 1.0)

        Dn = persist.tile([128, DTOK], MDT)   # normalized masked doc embeds
        Qn = persist.tile([128, QTOK], MDT)   # normalized query embeds
        mx = persist.tile([32, GROUPS * N_P], F32)
        out_sb = persist.tile([1, GROUPS * N_P], F32)
        if dma_only:
            nc.vector.memset(mx[:], 0.0)
            nc.vector.memset(out_sb[:], 0.0)

        def _once(_iv=None):
            # ---- query phase: project + L2-normalize 256 query tokens ----
            with tc.tile_pool(name="qps", bufs=1, space="PSUM") as qps:
                qx = qsb.tile([128, K_CH, QTOK], PDT, tag="qx")
                nc.sync.dma_start(out=qx[:], in_=qT3[:, :, :])
                qt_ps = qps.tile([128, QTOK], F32, tag="qt")
                for k in range(K_CH):
                    nc.tensor.matmul(qt_ps[:], wt[:, k, :], qx[:, k, :],
                                     start=(k == 0), stop=(k == K_CH - 1))
                qsq = qsb.tile([128, QTOK], SDT, tag="qsq")
                nc.scalar.square(qsq[:], qt_ps[:])
                qs2 = qps.tile([128, QTOK], F32, tag="qs2")
                nc.tensor.matmul(qs2[:], ones128[:], qsq[:],
                                 start=True, stop=True)
                qnrm = qsb.tile([128, QTOK], F32, tag="qnrm")
                nc.scalar.sqrt(qnrm[:], qs2[:])
                qnrm2 = qsb.tile([128, QTOK], F32, tag="qnrm2")
                nc.vector.tensor_scalar_max(qnrm2[:], qnrm[:], 1e-12)
                qinv = qsb.tile([128, QTOK], F32, tag="qinv")
                nc.vector.reciprocal(qinv[:], qnrm2[:])
                nc.vector.tensor_mul(Qn[:], qt_ps[:], qinv[:])

            # ---- doc loop: 8 groups x 4 tiles of 512 tokens ----
            with (
                tc.tile_pool(name="psA", bufs=2, space="PSUM") as psA,
                tc.tile_pool(name="psB", bufs=2, space="PSUM") as psB,
                tc.tile_pool(name="psS", bufs=1, space="PSUM") as psS,
            ):
                dx = None
                for g in range(GROUPS):
                    for j in range(TILES_PER_G):
                        t = g * TILES_PER_G + j
                        if t % dma_tile == 0:
                            dx = sb.tile([128, dma_tile, K_CH, TN], PDT,
                                         tag="dx")
                            nc.sync.dma_start(
                                out=dx[:],
                                in_=dT4[ds(t, dma_tile)].rearrange(
                                    "a p k t -> p a k t"))
                        dxs = dx[:, t % dma_tile, :, :]
                        if dma_only:
                            continue
                        dt_ps = psA.tile([128, TN], F32, tag="dt")
                        for k in range(K_CH):
                            nc.tensor.matmul(dt_ps[:], wt[:, k, :],
                                             dxs[:, k, :],
                                             start=(k == 0),
                                             stop=(k == K_CH - 1))
                        dsq = sb.tile([128, TN], SDT, tag="dsq")
                        nc.scalar.square(dsq[:], dt_ps[:])
                        s2 = psB.tile([128, TN], F32, tag="s2")
                        nc.tensor.matmul(s2[:], ones128[:], dsq[:],
                                         start=True, stop=False)
                        nc.tensor.matmul(s2[:], onesbig[:],
                                         amrow[0:1, ts(t, TN)],
                                         start=False, stop=True)
                        nrm = sb.tile([128, TN], F32, tag="nrm")
                        nc.scalar.sqrt(nrm[:], s2[:])
                        nrm2 = sb.tile([128, TN], F32, tag="nrm2")
                        nc.vector.tensor_scalar_max(nrm2[:], nrm[:], 1e-12)
                        inv = sb.tile([128, TN], F32, tag="inv")
                        nc.vector.reciprocal(inv[:], nrm2[:])
                        nc.vector.tensor_mul(Dn[:, ts(t, TN)], dt_ps[:], inv[:])

                    # ---- MaxSim for group g over its 2048 doc tokens ----
                    if dma_only:
                        continue
                    sim = psS.tile([32, N_P * L_D], F32, tag="sim")
                    for j in range(TILES_PER_G):
                        nc.tensor.matmul(
                            sim[:, ts(j, TN)],
                            Qn[:, ts(g, L_Q)],
                            Dn[:, ds(g * N_P * L_D + j * TN, TN)],
                            start=True, stop=True)
                    nc.vector.tensor_reduce(
                        mx[:, ts(g, N_P)],
                        sim[:].rearrange("p (n d) -> p n d", n=N_P),
                        axis=mybir.AxisListType.X, op=mybir.AluOpType.max)

            # ---- mean over the 32 queries (cross-partition via matmul) ----
            if dma_only:
                return
            with tc.tile_pool(name="psM", bufs=1, space="PSUM") as psM:
                mean_ps = psM.tile([1, GROUPS * N_P], F32, tag="mean")
                nc.tensor.matmul(mean_ps[:], ones32[:], mx[:],
                                 start=True, stop=True)
                nc.vector.tensor_scalar_mul(out_sb[:], mean_ps[:], 1.0 / L_Q)

        if loop_reps is not None:
            with tc.For_i(0, loop_reps, 1):
                _once()
        else:
            for _ in range(reps):
                _once()
        nc.sync.dma_start(out=out[:, :], in_=out_sb[:])

    nc.compile()
    return nc


def _shard_inputs(q_hidden, d_hidden, d_input_ids, skiplist, W):
    """Host-side shard + relayout. Returns per-core in_maps."""
    q_hidden = np.ascontiguousarray(q_hidden, dtype=np.float32)
    d_hidden = np.ascontiguousarray(d_hidden, dtype=np.float32)
    ids = np.asarray(d_input_ids)
    skip = np.asarray(skiplist)
    wT = np.ascontiguousarray(np.asarray(W, dtype=np.float32).T)  # [768, 128]

    masked = (ids == 0) | np.isin(ids, skip)           # True -> drop token
    anti = masked.astype(np.float32)                   # 1.0 where masked

    wH = np.ascontiguousarray(
        wT.reshape(K_CH, 128, OUT).transpose(1, 0, 2))           # [128, 6, 128]
    in_maps = []
    for c in range(N_CORES):
        dh = d_hidden[c * 64:(c + 1) * 64].reshape(-1, HID)      # [16384, 768]
        qh = q_hidden[c * GROUPS:(c + 1) * GROUPS].reshape(-1, HID)
        dH = np.ascontiguousarray(
            dh.reshape(D_TILES, TN, K_CH, 128).transpose(0, 3, 2, 1))
        qH = np.ascontiguousarray(
            qh.reshape(QTOK, K_CH, 128).transpose(2, 1, 0))      # [128, 6, 256]
        in_maps.append({
            "dT": dH,                           # [32, 128, 6, 512]
            "qT": qH,
            "wT": wH,
            "am": anti[c * 64:(c + 1) * 64].reshape(1, DTOK)
                      .astype(ml_dtypes.bfloat16),
        })
    return in_maps


_CACHED = {}


def _get_program(key=("default",), **kw):
    if key not in _CACHED:
        _CACHED[key] = _build_program(**kw)
    return _CACHED[key]


def kernel(q_hidden, d_hidden, d_input_ids, skiplist, W):
    nc = _get_program()
    in_maps = _shard_inputs(q_hidden, d_hidden, d_input_ids, skiplist, W)
    res = run_bass_kernel_spmd(nc, in_maps, list(range(N_CORES)))
    outs = [res.results[c]["out"].reshape(GROUPS, N_P) for c in range(N_CORES)]
    return np.concatenate(outs, axis=0)                # (64, 8)
